# revision 51
# baseline (speedup 1.0000x reference)
# Bass/Tile kernel for nn_LongTermAttention (continuous long-term attention
# with rectangular basis functions) on 8 Trainium2 NeuronCores.
#
# Mathematical rewrite (verified exact vs the reference):
#   * G = F^T (F F^T + ridge I)^{-1} for the rectangular basis on the padded
#     uniform grid collapses to G[l, n] = (1/4.5) * [l // 4 == n], so
#     Bc[b,n,e] = (1/4.5) * sum_{j<4} k[b,e,4n+j]  (4-wide sum pooling).
#   * psi on the integration grid is a one-hot selector, so the P=1000-point
#     continuous softmax reduces to basis space:
#       u_n   = exp(s_n)                      (|s| <= ~3, exp safe)
#       Z     = sum_n Wn_n u_n + w_last       (Wn = quadrature mass per basis)
#       ctx   = (u / Z) @ (Wn * values)
#     The max-subtraction in the reference cancels exactly.
#
# v2 performance structure:
#   * k is re-laid-out on host as kj[b, j, e, n] = k[b, e, 4n+j]; the 4-wide
#     pooling then happens INSIDE the DMA via SWDGE accum_op=add (4
#     accumulating transfers land k directly as pooled [e, n] tiles).
#     This removes all vector/gpsimd pooling work from the old design.
#   * q is transposed on host to qT[b, e, t] (no device/DMA transposes).
#   * exp is done in 2 big ACTIVATEs per (batch, head-pair) with no bias;
#     the quadrature mass Wn is folded into the values drain (a
#     tensor_scalar_mul that replaces the plain PSUM-drain copy) and the
#     Z column of values.
#   * ctx for one (batch, head-pair) accumulates into a single PSUM bank
#     [128, 4*65]; Z-normalization is batched (one add + one reciprocal
#     per head-pair, per-chain drain-muls split across vector/gpsimd).
#   * ~7 dummy warm-up matmuls at t=0 keep the PE HAM busy while the first
#     k tiles stream in, so real matmuls run at 2.4 GHz.
#
# Sharding: data-parallel over batch, 2 batches per core; weights replicated.

import numpy as np

B_FULL = 16
N_CORES = 8
B_PER = B_FULL // N_CORES  # 2
E = 512          # embed dim
L = 2048         # memory length
T = 256          # query length
N = 512          # basis count
H = 8            # heads
D = 64           # head dim
P_GRID = 1000    # integration points
RIDGE_C = 4.5    # F F^T diag (4.0) + ridge (0.5)
W_LAST = 1.0 / 1998.0

N_WARMUP_MM = 10

_CACHE = {}


def _host_constants(Wk, Wv):
    """Fold pooling normalization (1/4.5) and query scale (1/8) into the
    projection weights; build the per-basis quadrature-mass tile."""
    import ml_dtypes
    wk = (Wk.astype(np.float64) / (RIDGE_C * 8.0)).astype(ml_dtypes.bfloat16)
    wv = (Wv.astype(np.float64) / RIDGE_C).astype(ml_dtypes.bfloat16)
    p = np.arange(P_GRID)
    nmap = (512 * p) // 999
    w = np.full(P_GRID, 1.0 / 999.0)
    w[0] = w[-1] = 1.0 / 1998.0
    Wn = np.zeros(N)
    for i in range(P_GRID - 1):
        Wn[nmap[i]] += w[i]
    # win8[p, m*8 + h] = Wn[m*128 + p]  (8 identical cols per n-block m)
    win8 = np.repeat(Wn.reshape(4, 128, 1), 8, axis=2)  # [4, 128, 8]
    win8 = np.ascontiguousarray(win8.transpose(1, 0, 2).reshape(128, 32)
                                ).astype(np.float32)
    return wk, wv, win8


def _build_program():
    import concourse.bass as bass
    import concourse.mybir as mybir
    import concourse.tile as tile
    from concourse import bacc

    f32 = mybir.dt.float32
    bf16 = mybir.dt.bfloat16

    nc = bacc.Bacc(
        "TRN2",
        target_bir_lowering=False,
        debug=False,
        enable_asserts=False,
        num_devices=N_CORES,
    )

    kt_d = nc.dram_tensor("kt", [B_PER, L, E], bf16, kind="ExternalInput").ap()
    qt_d = nc.dram_tensor("qt", [B_PER, E, T], bf16, kind="ExternalInput").ap()
    wk_d = nc.dram_tensor("wk", [E, E], bf16, kind="ExternalInput").ap()
    wv_d = nc.dram_tensor("wv", [E, E], bf16, kind="ExternalInput").ap()
    win8_d = nc.dram_tensor("win8", [128, 32], f32, kind="ExternalInput").ap()
    pmat_d = nc.dram_tensor("pmat", [128, 32], bf16, kind="ExternalInput").ap()
    out_d = nc.dram_tensor("out", [B_PER, T, E], bf16, kind="ExternalOutput").ap()

    from contextlib import ExitStack
    with tile.TileContext(nc) as tc, ExitStack() as ctx:
        _kernel_body(ctx, tc, nc, mybir, kt_d, qt_d, wk_d, wv_d, win8_d,
                     pmat_d, out_d)

    nc.compile()
    return nc


def _kernel_body(ctx, tc, nc, mybir, kt_d, qt_d, wk_d, wv_d, win8_d,
                 pmat_d, out_d):
    f32 = mybir.dt.float32
    bf16 = mybir.dt.bfloat16
    Exp = mybir.ActivationFunctionType.Exp

    def pool(name, bufs, space="SBUF"):
        return ctx.enter_context(tc.tile_pool(name=name, bufs=bufs, space=space))

    consts = pool("consts", 1)
    kpool = pool("kpool", 8)
    plpool = pool("plpool", 8)
    qtpool = pool("qtpool", 8)
    ktpool = pool("ktpool", 8)
    vpool = pool("vpool", 8)
    upool = pool("upool", 4)
    zpool = pool("zpool", 8)
    opool = pool("opool", 4)

    ps_proj = pool("ps_proj", 2, "PSUM")   # [128,512] : 1 bank each
    ps_s = pool("ps_s", 2, "PSUM")         # [128,1024]: 2 banks each
    ps_c = pool("ps_c", 2, "PSUM")         # [128,260] : 1 bank each

    # ---- DMA emission order == ring order: wk then batch-0 k first (k
    # alternates rings for issue-rate; everything else issues from sync so
    # the scalar ENGINE queue, which also runs exp, stays light).
    wk_sb = consts.tile([128, 4 * 512], bf16, tag="wk")   # [e%128, (e//128)*512 + e']
    wv_sb = consts.tile([128, 4 * 512], bf16, tag="wv")
    win8_sb = consts.tile([128, 32], f32, tag="win8")
    pmat_sb = consts.tile([128, 32], bf16, tag="pmat")
    qT_b = [[qtpool.tile([128, T], bf16, tag="qT", name=f"qt{b}_{eb}")
             for eb in range(4)] for b in range(B_PER)]
    kt_b = [[kpool.tile([128, 4, 512], bf16, tag="k", name=f"k{b}_{ti}")
             for ti in range(4)] for b in range(B_PER)]
    pooled_b = [[plpool.tile([128, N], bf16, tag="pl", name=f"pl{b}_{kk}")
                 for kk in range(4)] for b in range(B_PER)]
    values_b = [[vpool.tile([128, 8 * 66], bf16, tag="values",
                            name=f"v{b}_{m}") for m in range(4)]
                for b in range(B_PER)]

    # DMA ring order: pmat + batch-0 k jump ahead of everything (they gate
    # the PE pooling); wk before qt (keys projection precedes scores).
    nc.sync.dma_start(pmat_sb[:], pmat_d[:])

    def emit_k(b, ti):
        # kT tile ti: [128 l-in-chunk, (chunk cc, e)] covering l-chunks
        # 4*ti .. 4*ti+3
        dma_eng = nc.sync if ti % 2 == 0 else nc.scalar
        dma_eng.dma_start(
            kt_b[b][ti][:, :, :],
            kt_d[b, ti * 512:(ti + 1) * 512, :].rearrange(
                "(cc p) e -> p cc e", p=128))

    for ti in range(4):
        emit_k(0, ti)
    nc.sync.dma_start(wk_sb[:].rearrange("p (kk e) -> p kk e", kk=4),
                      wk_d.rearrange("(kk p) e -> p kk e", p=128))
    nc.sync.dma_start(win8_sb[:], win8_d[:])
    # fill the values Z-columns (col 64 of each 66-block = Wn) early while
    # vector is idle; the drain later only writes cols 0:64
    for b in range(B_PER):
        for m in range(4):
            vv = values_b[b][m][:].rearrange("p (h c) -> p h c", c=66)
            nc.vector.tensor_copy(vv[:, :, 64], win8_sb[:, m * 8:(m + 1) * 8])
    for eb in range(4):
        nc.scalar.dma_start(qT_b[0][eb][:], qt_d[0, eb * 128:(eb + 1) * 128, :])
    nc.scalar.dma_start(wv_sb[:].rearrange("p (kk e) -> p kk e", kk=4),
                        wv_d.rearrange("(kk p) e -> p kk e", p=128))
    for ti in range(4):
        emit_k(1, ti)
    for eb in range(4):
        nc.sync.dma_start(qT_b[1][eb][:], qt_d[1, eb * 128:(eb + 1) * 128, :])

    # ---- PE warm-up: dummy matmuls with no deps keep HAM busy until the
    # first pooling matmuls become ready ----
    dummy = consts.tile([128, 512], bf16, tag="dummy")
    nc.vector.memset(dummy[:], 0.125)
    ps_w = ps_c.tile([128, 260], f32, tag="ps_c")
    for i in range(N_WARMUP_MM):
        nc.tensor.matmul(ps_w[:, 0:256], dummy[:, 0:128], dummy[:, 0:256],
                         start=True, stop=True, skip_group_check=True)

    # ---- pooling ON THE PE: pooled[e, c*32+n'] = sum_l kT[l, e] P[l, n'],
    # P[l_loc, n'] = [l_loc//4 == n'] (constant).  64 tiny matmuls per batch
    # in the otherwise-idle PE startup window; f32 PSUM accumulation.
    # Batch 0 uses the two (free until scores) ps_s slots in one pass;
    # batch 1 goes through the two ps_proj slots in two passes.
    def emit_pool_b0():
        pss = [ps_s.tile([128, 1024], f32, tag="ps_s", name=f"plps0_{i}")
               for i in range(2)]
        for ti in range(4):
            for cc in range(4):
                for m in range(4):
                    c0 = (m % 2) * 512 + (ti * 4 + cc) * 32
                    nc.tensor.matmul(
                        pss[m // 2][:, c0:c0 + 32],
                        kt_b[0][ti][:, cc, m * 128:(m + 1) * 128],
                        pmat_sb[:],
                        start=True, stop=True,
                        skip_group_check=True,
                    )
        # parallel halves: scalar takes m0/m1, vector m2/m3
        nc.scalar.copy(pooled_b[0][0][:], pss[0][:, 0:512])
        nc.vector.tensor_copy(pooled_b[0][2][:], pss[1][:, 0:512])
        nc.scalar.copy(pooled_b[0][1][:], pss[0][:, 512:1024])
        nc.vector.tensor_copy(pooled_b[0][3][:], pss[1][:, 512:1024])

    def emit_pool_b1():
        for pair in ((0, 1), (2, 3)):
            pss = {}
            for m in pair:
                pss[m] = ps_proj.tile([128, 512], f32, tag="ps_proj",
                                      name=f"plps1_{m}")
            for ti in range(4):
                for cc in range(4):
                    for m in pair:
                        nc.tensor.matmul(
                            pss[m][:, (ti * 4 + cc) * 32:
                                   (ti * 4 + cc) * 32 + 32],
                            kt_b[1][ti][:, cc, m * 128:(m + 1) * 128],
                            pmat_sb[:],
                            start=True, stop=True,
                            skip_group_check=True,
                        )
            for m in pair:
                # scalar: these land in batch-0's exp-stream gaps and keep
                # vector free for the keysT/values drains that follow
                nc.scalar.copy(pooled_b[1][m][:], pss[m][:])

    keysT_b = {}

    def emit_proj_keys(b):
        pooled = pooled_b[b]
        # ---- keysT = wk^T @ pooled  -> [e' (4x128 part), n=512] ----
        keysT = []
        for m in range(4):
            ps = ps_proj.tile([128, 512], f32, tag="ps_proj",
                              name=f"kps{b}_{m}")
            for kk in range(4):
                nc.tensor.matmul(
                    ps[:],
                    wk_sb[:, kk * 512 + m * 128: kk * 512 + (m + 1) * 128],
                    pooled[kk][:],
                    start=(kk == 0), stop=(kk == 3),
                )
            kt_sb = ktpool.tile([128, 512], bf16, tag="keysT",
                                name=f"keysT{b}_{m}")
            # batch-0: m0/m1 drain on scalar (pre-exp window) in parallel
            # with m2/m3 on vector, so scores hp0 unblocks earliest
            if b == 0 and m < 2:
                nc.scalar.copy(kt_sb[:], ps[:])
            else:
                nc.vector.tensor_copy(kt_sb[:], ps[:])
            keysT.append(kt_sb)
        keysT_b[b] = keysT

    def emit_proj_values(b):
        pooled = pooled_b[b]
        # ---- values' = Wn * (pooled^T @ wv) -> [n (4x128 part), 8*(64+Wn)] ----
        for m in range(4):
            ps = ps_proj.tile([128, 512], f32, tag="ps_proj",
                              name=f"vps{b}_{m}")
            for kk in range(4):
                nc.tensor.matmul(
                    ps[:],
                    pooled[kk][:, m * 128:(m + 1) * 128],
                    wv_sb[:, kk * 512:(kk + 1) * 512],
                    start=(kk == 0), stop=(kk == 3),
                )
            vv = values_b[b][m][:].rearrange("p (h c) -> p h c", c=66)
            nc.vector.tensor_scalar_mul(
                vv[:, :, 0:64],
                ps[:].rearrange("p (h d) -> p h d", d=64),
                win8_sb[:, m * 8:m * 8 + 1],
            )

    out_sbs_b = [[opool.tile([128, E], bf16, tag="out", name=f"out{b}_{mb}")
                  for mb in range(2)] for b in range(B_PER)]
    u_hp = {}

    def emit_scores_exp(b, hp):
        qT = qT_b[b]
        keysT = keysT_b[b]
        # scores: per h01 one [128, 4nb*256t] PSUM tile (2 banks); the h01
        # pair runs concurrently via PE row tiling (base partitions 0/64)
        # and drains to different banks.
        ps_h = [ps_s.tile([128, 1024], f32, tag="ps_s",
                          name=f"ps_s{b}_{hp}_{h01}") for h01 in range(2)]
        for nb in range(4):
            for h01 in range(2):
                nc.tensor.matmul(
                    ps_h[h01][:, nb * 256:(nb + 1) * 256],
                    keysT[hp][h01 * 64:(h01 + 1) * 64,
                              nb * 128:(nb + 1) * 128],
                    qT[hp][h01 * 64:(h01 + 1) * 64, :],
                    start=True, stop=True,
                    skip_group_check=True,
                )
        # exp: one big ACTIVATE per h01 (no bias; Wn lives in values')
        u_h = []
        for h01 in range(2):
            u = upool.tile([128, 1024], bf16, tag="u")
            nc.scalar.activation(u[:], ps_h[h01][:], Exp)
            u_h.append(u)
        u_hp[(b, hp)] = u_h

    def emit_ctx_norm(b, hp):
        values = values_b[b]
        out_sbs = out_sbs_b[b]
        u_h = u_hp[(b, hp)]
        # ctx (+Z at col 64 of each 65-block): 4 chains into one bank
        psc = ps_c.tile([128, 260], f32, tag="ps_c")
        for h01 in range(2):
            h = hp * 2 + h01
            for mb in range(2):
                c = (2 * h01 + mb) * 65
                for nb in range(4):
                    nc.tensor.matmul(
                        psc[:, c:c + 65],
                        u_h[h01][:, nb * 256 + mb * 128:
                                 nb * 256 + (mb + 1) * 128],
                        values[nb][:, h * 66:h * 66 + 65],
                        start=(nb == 0), stop=(nb == 3),
                        skip_group_check=True,
                    )
        # batched Z normalization for the whole head pair; one broadcast
        # tensor_tensor per t-block covers both heads
        pv = psc[:].rearrange("p (g c) -> p g c", c=65)
        z = zpool.tile([128, 4], f32, tag="z")
        nc.vector.tensor_scalar_add(z[:], pv[:, :, 64], W_LAST)
        zi = zpool.tile([128, 4], f32, tag="zi")
        nc.vector.reciprocal(zi[:], z[:])
        for mb in range(2):
            src = psc[:].rearrange("p (h01 two c) -> p h01 two c",
                                   two=2, c=65)[:, :, mb, 0:64]
            nc.vector.tensor_mul(
                out_sbs[mb][:, hp * 128:(hp + 1) * 128].rearrange(
                    "p (h01 dd) -> p h01 dd", dd=64),
                src,
                zi[:, mb::2].broadcast_to([128, 2, 64]),
            )
        if hp == 1:
            for mb in range(2):
                nc.sync.dma_start(out_d[b, mb * 128:(mb + 1) * 128, 0:256],
                                  out_sbs[mb][:, 0:256])
        if hp == 3:
            for mb in range(2):
                nc.sync.dma_start(out_d[b, mb * 128:(mb + 1) * 128, 256:512],
                                  out_sbs[mb][:, 256:512])

    # Orchestration.  Emission order is the static per-engine queue order,
    # and engine queues are in-order: an instruction waiting on a semaphore
    # blocks everything behind it on that engine.  So batch-1's pool/proj
    # chain (gated by its late DMA) is emitted BETWEEN batch-0's head-pairs:
    # late enough not to block batch-0's score/ctx matmuls, early enough
    # that its vector drains outrank batch-0's tail normalization and the
    # batch-1 exp stream can start the moment batch-0's ends.
    emit_pool_b0()
    emit_proj_keys(0)
    emit_scores_exp(0, 0)
    emit_scores_exp(0, 1)
    emit_proj_values(0)
    emit_ctx_norm(0, 0)
    emit_scores_exp(0, 2)
    emit_ctx_norm(0, 1)
    emit_pool_b1()
    emit_scores_exp(0, 3)
    emit_ctx_norm(0, 2)
    emit_proj_keys(1)
    emit_ctx_norm(0, 3)
    emit_proj_values(1)
    for hp in range(4):
        emit_scores_exp(1, hp)
        emit_ctx_norm(1, hp)


def _get_program():
    if "nc" not in _CACHE:
        _CACHE["nc"] = _build_program()
    return _CACHE["nc"]


def make_in_maps(k, q, Wk, Wv):
    import ml_dtypes
    wk, wv, win8 = _host_constants(Wk, Wv)
    k16 = np.asarray(k).astype(ml_dtypes.bfloat16)
    # kt[b, l, e] = k[b, e, l]  (pooling runs on the PE as kT_chunk.T @ P)
    kt = np.ascontiguousarray(k16.transpose(0, 2, 1))
    qt = np.ascontiguousarray(
        np.asarray(q).astype(ml_dtypes.bfloat16).transpose(0, 2, 1))
    # P[l_loc, n'] = 1 iff l_loc // 4 == n'
    pmat = (np.arange(128)[:, None] // 4 ==
            np.arange(32)[None, :]).astype(ml_dtypes.bfloat16)
    in_maps = []
    for c in range(N_CORES):
        in_maps.append({
            "kt": np.ascontiguousarray(kt[c * B_PER:(c + 1) * B_PER]),
            "qt": np.ascontiguousarray(qt[c * B_PER:(c + 1) * B_PER]),
            "wk": wk,
            "wv": wv,
            "win8": win8,
            "pmat": pmat,
        })
    return in_maps


def kernel(k, q, Wk, Wv):
    from concourse.bass_utils import run_bass_kernel_spmd

    in_maps = make_in_maps(k, q, Wk, Wv)
    nc = _get_program()
    res = run_bass_kernel_spmd(nc, in_maps, core_ids=list(range(N_CORES)))
    return np.concatenate(
        [res.results[c]["out"].astype(np.float32) for c in range(N_CORES)],
        axis=0)


# revision 52
# speedup vs baseline: 1.0431x; 1.0431x over previous
# Bass/Tile kernel for nn_LongTermAttention (continuous long-term attention
# with rectangular basis functions) on 8 Trainium2 NeuronCores.
#
# Mathematical rewrite (verified exact vs the reference):
#   * G = F^T (F F^T + ridge I)^{-1} for the rectangular basis on the padded
#     uniform grid collapses to G[l, n] = (1/4.5) * [l // 4 == n], so
#     Bc[b,n,e] = (1/4.5) * sum_{j<4} k[b,e,4n+j]  (4-wide sum pooling).
#   * psi on the integration grid is a one-hot selector, so the P=1000-point
#     continuous softmax reduces to basis space:
#       u_n   = exp(s_n)                      (|s| <= ~3, exp safe)
#       Z     = sum_n Wn_n u_n + w_last       (Wn = quadrature mass per basis)
#       ctx   = (u / Z) @ (Wn * values)
#     The max-subtraction in the reference cancels exactly.
#
# v2 performance structure:
#   * k is re-laid-out on host as kj[b, j, e, n] = k[b, e, 4n+j]; the 4-wide
#     pooling then happens INSIDE the DMA via SWDGE accum_op=add (4
#     accumulating transfers land k directly as pooled [e, n] tiles).
#     This removes all vector/gpsimd pooling work from the old design.
#   * q is transposed on host to qT[b, e, t] (no device/DMA transposes).
#   * exp is done in 2 big ACTIVATEs per (batch, head-pair) with no bias;
#     the quadrature mass Wn is folded into the values drain (a
#     tensor_scalar_mul that replaces the plain PSUM-drain copy) and the
#     Z column of values.
#   * ctx for one (batch, head-pair) accumulates into a single PSUM bank
#     [128, 4*65]; Z-normalization is batched (one add + one reciprocal
#     per head-pair, per-chain drain-muls split across vector/gpsimd).
#   * ~7 dummy warm-up matmuls at t=0 keep the PE HAM busy while the first
#     k tiles stream in, so real matmuls run at 2.4 GHz.
#
# Sharding: data-parallel over batch, 2 batches per core; weights replicated.

import numpy as np

B_FULL = 16
N_CORES = 8
B_PER = B_FULL // N_CORES  # 2
E = 512          # embed dim
L = 2048         # memory length
T = 256          # query length
N = 512          # basis count
H = 8            # heads
D = 64           # head dim
P_GRID = 1000    # integration points
RIDGE_C = 4.5    # F F^T diag (4.0) + ridge (0.5)
W_LAST = 1.0 / 1998.0

N_WARMUP_MM = 10

_CACHE = {}


def _host_constants(Wk, Wv):
    """Fold pooling normalization (1/4.5) and query scale (1/8) into the
    projection weights; build the per-basis quadrature-mass tile."""
    import ml_dtypes
    wk = (Wk.astype(np.float64) / (RIDGE_C * 8.0)).astype(ml_dtypes.bfloat16)
    wv = (Wv.astype(np.float64) / RIDGE_C).astype(ml_dtypes.bfloat16)
    p = np.arange(P_GRID)
    nmap = (512 * p) // 999
    w = np.full(P_GRID, 1.0 / 999.0)
    w[0] = w[-1] = 1.0 / 1998.0
    Wn = np.zeros(N)
    for i in range(P_GRID - 1):
        Wn[nmap[i]] += w[i]
    # win8[p, m*8 + h] = Wn[m*128 + p]  (8 identical cols per n-block m)
    win8 = np.repeat(Wn.reshape(4, 128, 1), 8, axis=2)  # [4, 128, 8]
    win8 = np.ascontiguousarray(win8.transpose(1, 0, 2).reshape(128, 32)
                                ).astype(np.float32)
    return wk, wv, win8


def _build_program():
    import concourse.bass as bass
    import concourse.mybir as mybir
    import concourse.tile as tile
    from concourse import bacc

    f32 = mybir.dt.float32
    bf16 = mybir.dt.bfloat16

    nc = bacc.Bacc(
        "TRN2",
        target_bir_lowering=False,
        debug=False,
        enable_asserts=False,
        num_devices=N_CORES,
    )

    kt_d = nc.dram_tensor("kt", [B_PER, L, E], bf16, kind="ExternalInput").ap()
    qt_d = nc.dram_tensor("qt", [B_PER, E, T], bf16, kind="ExternalInput").ap()
    wk_d = nc.dram_tensor("wk", [E, E], bf16, kind="ExternalInput").ap()
    wv_d = nc.dram_tensor("wv", [E, E], bf16, kind="ExternalInput").ap()
    win8_d = nc.dram_tensor("win8", [128, 32], f32, kind="ExternalInput").ap()
    pmat_d = nc.dram_tensor("pmat", [128, 32], bf16, kind="ExternalInput").ap()
    out_d = nc.dram_tensor("out", [B_PER, T, E], bf16, kind="ExternalOutput").ap()

    from contextlib import ExitStack
    with tile.TileContext(nc) as tc, ExitStack() as ctx:
        _kernel_body(ctx, tc, nc, mybir, kt_d, qt_d, wk_d, wv_d, win8_d,
                     pmat_d, out_d)

    nc.compile()
    return nc


def _kernel_body(ctx, tc, nc, mybir, kt_d, qt_d, wk_d, wv_d, win8_d,
                 pmat_d, out_d):
    f32 = mybir.dt.float32
    bf16 = mybir.dt.bfloat16
    Exp = mybir.ActivationFunctionType.Exp

    def pool(name, bufs, space="SBUF"):
        return ctx.enter_context(tc.tile_pool(name=name, bufs=bufs, space=space))

    consts = pool("consts", 1)
    kpool = pool("kpool", 8)
    plpool = pool("plpool", 8)
    qtpool = pool("qtpool", 8)
    ktpool = pool("ktpool", 8)
    vpool = pool("vpool", 8)
    upool = pool("upool", 4)
    zpool = pool("zpool", 8)
    opool = pool("opool", 4)

    ps_proj = pool("ps_proj", 2, "PSUM")   # [128,512] : 1 bank each
    ps_s = pool("ps_s", 2, "PSUM")         # [128,1024]: 2 banks each
    ps_c = pool("ps_c", 2, "PSUM")         # [128,260] : 1 bank each

    # ---- DMA emission order == ring order: wk then batch-0 k first (k
    # alternates rings for issue-rate; everything else issues from sync so
    # the scalar ENGINE queue, which also runs exp, stays light).
    wk_sb = consts.tile([128, 4 * 512], bf16, tag="wk")   # [e%128, (e//128)*512 + e']
    wv_sb = consts.tile([128, 4 * 512], bf16, tag="wv")
    win8_sb = consts.tile([128, 32], f32, tag="win8")
    pmat_sb = consts.tile([128, 32], bf16, tag="pmat")
    qT_b = [[qtpool.tile([128, T], bf16, tag="qT", name=f"qt{b}_{eb}")
             for eb in range(4)] for b in range(B_PER)]
    kt_b = [[kpool.tile([128, 4, 512], bf16, tag="k", name=f"k{b}_{ti}")
             for ti in range(4)] for b in range(B_PER)]
    pooled_b = [[plpool.tile([128, N], bf16, tag="pl", name=f"pl{b}_{kk}")
                 for kk in range(4)] for b in range(B_PER)]
    values_b = [[vpool.tile([128, 8 * 66], bf16, tag="values",
                            name=f"v{b}_{m}") for m in range(4)]
                for b in range(B_PER)]

    # DMA ring order: pmat + batch-0 k jump ahead of everything (they gate
    # the PE pooling); wk before qt (keys projection precedes scores).
    nc.sync.dma_start(pmat_sb[:], pmat_d[:])

    def emit_k(b, ti):
        # kT tile ti: [128 l-in-chunk, (chunk cc, e)] covering l-chunks
        # 4*ti .. 4*ti+3
        dma_eng = nc.sync if ti % 2 == 0 else nc.scalar
        dma_eng.dma_start(
            kt_b[b][ti][:, :, :],
            kt_d[b, ti * 512:(ti + 1) * 512, :].rearrange(
                "(cc p) e -> p cc e", p=128))

    for ti in range(4):
        emit_k(0, ti)
    nc.sync.dma_start(wk_sb[:].rearrange("p (kk e) -> p kk e", kk=4),
                      wk_d.rearrange("(kk p) e -> p kk e", p=128))
    nc.sync.dma_start(win8_sb[:], win8_d[:])
    # fill the values Z-columns (col 64 of each 66-block = Wn) early while
    # vector is idle; the drain later only writes cols 0:64
    for b in range(B_PER):
        for m in range(4):
            vv = values_b[b][m][:].rearrange("p (h c) -> p h c", c=66)
            nc.vector.tensor_copy(vv[:, :, 64], win8_sb[:, m * 8:(m + 1) * 8])
    for eb in range(4):
        nc.scalar.dma_start(qT_b[0][eb][:], qt_d[0, eb * 128:(eb + 1) * 128, :])
    nc.scalar.dma_start(wv_sb[:].rearrange("p (kk e) -> p kk e", kk=4),
                        wv_d.rearrange("(kk p) e -> p kk e", p=128))
    for ti in range(4):
        emit_k(1, ti)
    for eb in range(4):
        nc.sync.dma_start(qT_b[1][eb][:], qt_d[1, eb * 128:(eb + 1) * 128, :])

    # ---- PE warm-up: dummy matmuls with no deps keep HAM busy until the
    # first pooling matmuls become ready ----
    dummy = consts.tile([128, 512], bf16, tag="dummy")
    nc.vector.memset(dummy[:], 0.125)
    ps_w = ps_c.tile([128, 260], f32, tag="ps_c")
    for i in range(N_WARMUP_MM):
        nc.tensor.matmul(ps_w[:, 0:256], dummy[:, 0:128], dummy[:, 0:256],
                         start=True, stop=True, skip_group_check=True)

    # ---- pooling ON THE PE: pooled[e, c*32+n'] = sum_l kT[l, e] P[l, n'],
    # P[l_loc, n'] = [l_loc//4 == n'] (constant).  64 tiny matmuls per batch
    # in the otherwise-idle PE startup window; f32 PSUM accumulation.
    # Batch 0 uses the two (free until scores) ps_s slots in one pass;
    # batch 1 goes through the two ps_proj slots in two passes.
    def emit_pool_b0():
        pss = [ps_s.tile([128, 1024], f32, tag="ps_s", name=f"plps0_{i}")
               for i in range(2)]
        for ti in range(4):
            for cc in range(4):
                for m in range(4):
                    c0 = (m % 2) * 512 + (ti * 4 + cc) * 32
                    nc.tensor.matmul(
                        pss[m // 2][:, c0:c0 + 32],
                        kt_b[0][ti][:, cc, m * 128:(m + 1) * 128],
                        pmat_sb[:],
                        start=True, stop=True,
                        skip_group_check=True,
                    )
        # parallel halves: scalar takes m0/m1, vector m2/m3
        nc.scalar.copy(pooled_b[0][0][:], pss[0][:, 0:512])
        nc.vector.tensor_copy(pooled_b[0][2][:], pss[1][:, 0:512])
        nc.scalar.copy(pooled_b[0][1][:], pss[0][:, 512:1024])
        nc.vector.tensor_copy(pooled_b[0][3][:], pss[1][:, 512:1024])

    def emit_pool_b1():
        for pair in ((0, 1), (2, 3)):
            pss = {}
            for m in pair:
                pss[m] = ps_proj.tile([128, 512], f32, tag="ps_proj",
                                      name=f"plps1_{m}")
            for ti in range(4):
                for cc in range(4):
                    for m in pair:
                        nc.tensor.matmul(
                            pss[m][:, (ti * 4 + cc) * 32:
                                   (ti * 4 + cc) * 32 + 32],
                            kt_b[1][ti][:, cc, m * 128:(m + 1) * 128],
                            pmat_sb[:],
                            start=True, stop=True,
                            skip_group_check=True,
                        )
            for m in pair:
                nc.vector.tensor_copy(pooled_b[1][m][:], pss[m][:])

    keysT_b = {}

    def emit_proj_keys(b):
        pooled = pooled_b[b]
        # ---- keysT = wk^T @ pooled  -> [e' (4x128 part), n=512] ----
        keysT = []
        for m in range(4):
            ps = ps_proj.tile([128, 512], f32, tag="ps_proj",
                              name=f"kps{b}_{m}")
            for kk in range(4):
                nc.tensor.matmul(
                    ps[:],
                    wk_sb[:, kk * 512 + m * 128: kk * 512 + (m + 1) * 128],
                    pooled[kk][:],
                    start=(kk == 0), stop=(kk == 3),
                )
            kt_sb = ktpool.tile([128, 512], bf16, tag="keysT",
                                name=f"keysT{b}_{m}")
            # batch-0: m0/m1 drain on scalar (pre-exp window) in parallel
            # with m2/m3 on vector, so scores hp0 unblocks earliest
            if b == 0 and m < 2:
                nc.scalar.copy(kt_sb[:], ps[:])
            else:
                nc.vector.tensor_copy(kt_sb[:], ps[:])
            keysT.append(kt_sb)
        keysT_b[b] = keysT

    def emit_proj_values(b):
        pooled = pooled_b[b]
        # ---- values' = Wn * (pooled^T @ wv) -> [n (4x128 part), 8*(64+Wn)] ----
        for m in range(4):
            ps = ps_proj.tile([128, 512], f32, tag="ps_proj",
                              name=f"vps{b}_{m}")
            for kk in range(4):
                nc.tensor.matmul(
                    ps[:],
                    pooled[kk][:, m * 128:(m + 1) * 128],
                    wv_sb[:, kk * 512:(kk + 1) * 512],
                    start=(kk == 0), stop=(kk == 3),
                )
            vv = values_b[b][m][:].rearrange("p (h c) -> p h c", c=66)
            nc.vector.tensor_scalar_mul(
                vv[:, :, 0:64],
                ps[:].rearrange("p (h d) -> p h d", d=64),
                win8_sb[:, m * 8:m * 8 + 1],
            )

    out_sbs_b = [[opool.tile([128, E], bf16, tag="out", name=f"out{b}_{mb}")
                  for mb in range(2)] for b in range(B_PER)]
    u_hp = {}

    def emit_scores_exp(b, hp):
        qT = qT_b[b]
        keysT = keysT_b[b]
        # scores: per h01 one [128, 4nb*256t] PSUM tile (2 banks); the h01
        # pair runs concurrently via PE row tiling (base partitions 0/64)
        # and drains to different banks.
        ps_h = [ps_s.tile([128, 1024], f32, tag="ps_s",
                          name=f"ps_s{b}_{hp}_{h01}") for h01 in range(2)]
        for nb in range(4):
            for h01 in range(2):
                nc.tensor.matmul(
                    ps_h[h01][:, nb * 256:(nb + 1) * 256],
                    keysT[hp][h01 * 64:(h01 + 1) * 64,
                              nb * 128:(nb + 1) * 128],
                    qT[hp][h01 * 64:(h01 + 1) * 64, :],
                    start=True, stop=True,
                    skip_group_check=True,
                )
        # exp: one big ACTIVATE per h01 (no bias; Wn lives in values')
        u_h = []
        for h01 in range(2):
            u = upool.tile([128, 1024], bf16, tag="u")
            nc.scalar.activation(u[:], ps_h[h01][:], Exp)
            u_h.append(u)
        u_hp[(b, hp)] = u_h

    def emit_ctx_norm(b, hp):
        values = values_b[b]
        out_sbs = out_sbs_b[b]
        u_h = u_hp[(b, hp)]
        # ctx (+Z at col 64 of each 65-block): 4 chains into one bank
        psc = ps_c.tile([128, 260], f32, tag="ps_c")
        for h01 in range(2):
            h = hp * 2 + h01
            for mb in range(2):
                c = (2 * h01 + mb) * 65
                for nb in range(4):
                    nc.tensor.matmul(
                        psc[:, c:c + 65],
                        u_h[h01][:, nb * 256 + mb * 128:
                                 nb * 256 + (mb + 1) * 128],
                        values[nb][:, h * 66:h * 66 + 65],
                        start=(nb == 0), stop=(nb == 3),
                        skip_group_check=True,
                    )
        # batched Z normalization for the whole head pair; one broadcast
        # tensor_tensor per t-block covers both heads
        pv = psc[:].rearrange("p (g c) -> p g c", c=65)
        z = zpool.tile([128, 4], f32, tag="z")
        nc.vector.tensor_scalar_add(z[:], pv[:, :, 64], W_LAST)
        zi = zpool.tile([128, 4], f32, tag="zi")
        nc.vector.reciprocal(zi[:], z[:])
        for mb in range(2):
            src = psc[:].rearrange("p (h01 two c) -> p h01 two c",
                                   two=2, c=65)[:, :, mb, 0:64]
            nc.vector.tensor_mul(
                out_sbs[mb][:, hp * 128:(hp + 1) * 128].rearrange(
                    "p (h01 dd) -> p h01 dd", dd=64),
                src,
                zi[:, mb::2].broadcast_to([128, 2, 64]),
            )
        if hp == 1:
            for mb in range(2):
                nc.sync.dma_start(out_d[b, mb * 128:(mb + 1) * 128, 0:256],
                                  out_sbs[mb][:, 0:256])
        if hp == 3:
            for mb in range(2):
                nc.sync.dma_start(out_d[b, mb * 128:(mb + 1) * 128, 256:512],
                                  out_sbs[mb][:, 256:512])

    # Orchestration.  Emission order is the static per-engine queue order,
    # and engine queues are in-order: an instruction waiting on a semaphore
    # blocks everything behind it on that engine.  So batch-1's pool/proj
    # chain (gated by its late DMA) is emitted BETWEEN batch-0's head-pairs:
    # late enough not to block batch-0's score/ctx matmuls, early enough
    # that its vector drains outrank batch-0's tail normalization and the
    # batch-1 exp stream can start the moment batch-0's ends.
    emit_pool_b0()
    emit_proj_keys(0)
    emit_scores_exp(0, 0)
    emit_scores_exp(0, 1)
    emit_proj_values(0)
    emit_ctx_norm(0, 0)
    emit_scores_exp(0, 2)
    emit_ctx_norm(0, 1)
    emit_pool_b1()
    emit_scores_exp(0, 3)
    emit_ctx_norm(0, 2)
    emit_proj_keys(1)
    emit_ctx_norm(0, 3)
    emit_proj_values(1)
    for hp in range(4):
        emit_scores_exp(1, hp)
        emit_ctx_norm(1, hp)


def _get_program():
    if "nc" not in _CACHE:
        _CACHE["nc"] = _build_program()
    return _CACHE["nc"]


def make_in_maps(k, q, Wk, Wv):
    import ml_dtypes
    wk, wv, win8 = _host_constants(Wk, Wv)
    k16 = np.asarray(k).astype(ml_dtypes.bfloat16)
    # kt[b, l, e] = k[b, e, l]  (pooling runs on the PE as kT_chunk.T @ P)
    kt = np.ascontiguousarray(k16.transpose(0, 2, 1))
    qt = np.ascontiguousarray(
        np.asarray(q).astype(ml_dtypes.bfloat16).transpose(0, 2, 1))
    # P[l_loc, n'] = 1 iff l_loc // 4 == n'
    pmat = (np.arange(128)[:, None] // 4 ==
            np.arange(32)[None, :]).astype(ml_dtypes.bfloat16)
    in_maps = []
    for c in range(N_CORES):
        in_maps.append({
            "kt": np.ascontiguousarray(kt[c * B_PER:(c + 1) * B_PER]),
            "qt": np.ascontiguousarray(qt[c * B_PER:(c + 1) * B_PER]),
            "wk": wk,
            "wv": wv,
            "win8": win8,
            "pmat": pmat,
        })
    return in_maps


def kernel(k, q, Wk, Wv):
    from concourse.bass_utils import run_bass_kernel_spmd

    in_maps = make_in_maps(k, q, Wk, Wv)
    nc = _get_program()
    res = run_bass_kernel_spmd(nc, in_maps, core_ids=list(range(N_CORES)))
    return np.concatenate(
        [res.results[c]["out"].astype(np.float32) for c in range(N_CORES)],
        axis=0)
